# revision 20
# baseline (speedup 1.0000x reference)
"""GQA multi-head attention (RoPE + tanh softcap + causal mask) on 8 TRN2 cores.

Sharding: tensor-parallel over the 8 kv-head groups (1 kv head + its 4 q heads
per core).  Each core computes its Q/K/V projections from the full hidden
states, runs attention for its 4 q heads, and produces a partial output
through its row-slice of Wo; the host sums the 8 partials.

v2 layout/engine strategy (per core):
  - Q/K path stays float32r end-to-end (logits accurate to ~1e-4); the V/Wo
    path runs in bf16 (V, attention weights post-exp, attnT, Wo) which is
    insensitive at the 2e-2 tolerance and halves SBUF/DMA on that side.
  - logits are computed transposed ([kcol, qrow]); tanh softcap bounds them
    to +-30 so softmax needs no running max.  Diagonal (causally mixed)
    128-col chunks are computed on the visible qrow sub-range only, and the
    {0,1} mask multiply shrinks to the 128-wide partial strip.
  - softmax denominator: bf16 row-sums accumulated on the vector engine,
    reduced across partitions with a ones-matmul, inverted with the fast
    custom-DVE reciprocal (~5x faster than InstReciprocal), broadcast via
    gpsimd, applied on the vector engine.
  - DMA queues are specialized: hidden states stream on the sync queue
    (HWDGE) alone; RoPE rotate-half copies and mask tiles go on the scalar
    queue; weights load on gpsimd/scalar/vector in d-chunk order so the
    first projection matmuls start within a few us.
  - Wo PSUM evictions alternate vector/scalar engines to balance load.
"""

import numpy as np

S, D, DH = 2048, 4096, 128
HQ, HKV = 32, 8
G = HQ // HKV            # q heads per core
N_CORES = 8
MULT = 0.08838834764831845
SOFTCAP = 30.0
ROPE_BASE = 10000.0
BLK = 512                # seq block (matmul moving-dim max for 4-byte dtypes)
NB = S // BLK            # 4 seq blocks
NCH = S // 128           # 16 kcol chunks
NDC = D // 128           # 32 contraction chunks for projections

_CACHE = {}


def _classify_mask(mask):
    """Per (qblock, kchunk) in the transposed [kcol, qrow-local] layout:
    skip (all masked), plain (all visible), or mixed.  Mixed chunks carry
    (lo, pe, off): visible qrow cols form the suffix [lo, 512); cols in
    [lo, pe) are partially masked (mask tile at offset `off` in the packed
    [128, total_width] bf16 mask tensor); cols [pe, 512) are fully visible.
    """
    m = np.asarray(mask).reshape(S, S)
    active = []
    mtiles = []
    off = 0
    for n in range(NB):
        rows = m[n * BLK:(n + 1) * BLK]
        lst = []
        for c in range(NCH):
            sub = rows[:, c * 128:(c + 1) * 128]   # [qrow-local, kcol]
            vis = sub.any(axis=1)
            if not vis.any():
                continue
            full = sub.all(axis=1)
            if full.all():
                lst.append((c, 0, 0, -1))
                continue
            lo = int(np.argmax(vis))
            # visibility must be a suffix, and full-visibility a suffix of it
            assert vis[lo:].all(), "mask rows must be a suffix per chunk"
            if full[lo:].any():
                pe = lo + int(np.argmax(full[lo:]))
                assert full[pe:].all(), "full rows must form a suffix"
            else:
                pe = BLK
            lst.append((c, lo, pe, off))
            mtiles.append(np.ascontiguousarray(sub[lo:pe, :].T))  # [128, pe-lo]
            off += pe - lo
        assert lst and lst[0][0] == 0 and lst[0][1] == 0, \
            "first active chunk must cover qrow col 0"
        active.append(tuple(lst))
    return tuple(active), mtiles


def _build(active, total_w):
    import concourse.bacc as bacc
    import concourse.mybir as mybir
    from concourse import tile
    from concourse.masks import make_identity
    from contextlib import ExitStack

    fp32 = mybir.dt.float32
    f32r = mybir.dt.float32r
    bf16 = mybir.dt.bfloat16
    AF = mybir.ActivationFunctionType

    nc = bacc.Bacc("TRN2", target_bir_lowering=False, debug=False,
                   enable_asserts=True, num_devices=N_CORES)
    hsT = nc.dram_tensor("hsT", [D, S], f32r, kind="ExternalInput").ap()
    wq = nc.dram_tensor("wq", [D, G * DH], f32r, kind="ExternalInput").ap()
    wk = nc.dram_tensor("wk", [D, DH], f32r, kind="ExternalInput").ap()
    wv = nc.dram_tensor("wv", [D, DH], f32r, kind="ExternalInput").ap()
    wo = nc.dram_tensor("wo", [G * DH, D], bf16, kind="ExternalInput").ap()
    cosT = nc.dram_tensor("cosT", [DH, S], fp32, kind="ExternalInput").ap()
    sinT = nc.dram_tensor("sinT", [DH, S], fp32, kind="ExternalInput").ap()
    maskm = (nc.dram_tensor("maskm", [128, total_w], bf16,
                            kind="ExternalInput").ap() if total_w else None)
    out = nc.dram_tensor("out", [S, D], fp32, kind="ExternalOutput").ap()

    with tile.TileContext(nc) as tc, ExitStack() as top:
        persist = top.enter_context(tc.tile_pool(name="persist", bufs=1))
        qT = [[persist.tile([DH, BLK], f32r, tag=f"qT{h}_{n}",
                            name=f"qT{h}_{n}") for n in range(NB)]
              for h in range(G)]
        kT = [persist.tile([DH, BLK], f32r, tag=f"kT{n}", name=f"kT{n}")
              for n in range(NB)]
        vnat = [persist.tile([128, BLK], bf16, tag=f"vnat{n}",
                             name=f"vnat{n}") for n in range(NB)]

        # ---------------- Phase 1: QKV projections + RoPE ----------------
        with ExitStack() as ph1:
            const = ph1.enter_context(tc.tile_pool(name="p1const", bufs=1))
            wq_sb = const.tile([128, NDC, G * DH], f32r, tag="wq")
            wk_sb = const.tile([128, NDC, DH], f32r, tag="wk")
            wv_sb = const.tile([128, NDC, DH], f32r, tag="wv")
            cos_sb = const.tile([DH, S], fp32, tag="cos")
            sin_sb = const.tile([DH, S], fp32, tag="sin")
            identb = const.tile([128, 128], bf16, tag="identb")
            wq_r = wq.rearrange("(c p) m -> p c m", p=128)
            wk_r = wk.rearrange("(c p) m -> p c m", p=128)
            wv_r = wv.rearrange("(c p) m -> p c m", p=128)
            # d-chunk-ordered weight loads on three queues so the d=0
            # matmuls can start within a few us.
            for g in range(8):
                gs = slice(g * 4, (g + 1) * 4)
                nc.scalar.dma_start(wq_sb[:, gs, :], wq_r[:, gs, :])
                nc.scalar.dma_start(wk_sb[:, gs, :], wk_r[:, gs, :])
                nc.scalar.dma_start(wv_sb[:, gs, :], wv_r[:, gs, :])
            nc.gpsimd.dma_start(cos_sb[:], cosT[:])
            nc.gpsimd.dma_start(sin_sb[:], sinT[:])
            make_identity(nc, identb[:])

            hsp = ph1.enter_context(tc.tile_pool(name="hs", bufs=10))
            pps = ph1.enter_context(
                tc.tile_pool(name="projps", bufs=7, space="PSUM"))
            rawp = ph1.enter_context(tc.tile_pool(name="raw", bufs=3))
            rotp = ph1.enter_context(tc.tile_pool(name="rot", bufs=3))
            tmpp = ph1.enter_context(tc.tile_pool(name="tmp", bufs=3))
            vtp = ph1.enter_context(tc.tile_pool(name="vtp", bufs=2))
            tps = ph1.enter_context(
                tc.tile_pool(name="tps", bufs=1, space="PSUM"))

            for n in range(NB):
                sl = slice(n * BLK, (n + 1) * BLK)
                ps = [pps.tile([128, BLK], fp32, tag="projps", name="projps")
                      for _ in range(G + 2)]
                for d in range(NDC):
                    hs_t = hsp.tile([128, BLK], f32r, tag="hs")
                    nc.sync.dma_start(hs_t[:], hsT[d * 128:(d + 1) * 128, sl])
                    for h in range(G):
                        nc.tensor.matmul(ps[h][:],
                                         wq_sb[:, d, h * DH:(h + 1) * DH],
                                         hs_t[:], start=(d == 0),
                                         stop=(d == NDC - 1))
                    nc.tensor.matmul(ps[G][:], wk_sb[:, d, :], hs_t[:],
                                     start=(d == 0), stop=(d == NDC - 1))
                    nc.tensor.matmul(ps[G + 1][:], wv_sb[:, d, :], hs_t[:],
                                     start=(d == 0), stop=(d == NDC - 1))
                # V first: evict to bf16, PE-transpose 128-chunks into one
                # PSUM bank, evict once to vnat[n] ([kcol, dh] per chunk).
                # V-first keeps the PE busy with transposes and frees its
                # bank before the scalar/vector RoPE chain runs.
                vt = vtp.tile([128, BLK], bf16, tag="vt")
                nc.scalar.copy(vt[:], ps[G + 1][:])
                tp = tps.tile([128, BLK], bf16, tag="tp")
                for j in range(BLK // 128):
                    nc.tensor.matmul(tp[:, j * 128:(j + 1) * 128],
                                     vt[:, j * 128:(j + 1) * 128], identb[:],
                                     is_transpose=True, start=True, stop=True,
                                     skip_group_check=True)
                nc.vector.tensor_copy(vnat[n][:], tp[:])
                # RoPE on q heads and k: evict PSUM, then rotate-half via
                # scalar-queue SBUF-SBUF DMA (keeps the sync queue for hs).
                for i, dest in enumerate([qT[h][n] for h in range(G)]
                                         + [kT[n]]):
                    raw = rawp.tile([128, BLK], fp32, tag="raw")
                    if i % 2 == 0:
                        nc.scalar.copy(raw[:], ps[i][:])
                    else:
                        nc.vector.tensor_copy(raw[:], ps[i][:])
                    rot = rotp.tile([128, BLK], fp32, tag="rot")
                    nc.scalar.dma_start(rot[0:64, :], raw[64:128, :])
                    nc.scalar.dma_start(rot[64:128, :], raw[0:64, :])
                    tmp = tmpp.tile([128, BLK], fp32, tag="tmp")
                    nc.vector.tensor_mul(tmp[:], raw[:], cos_sb[:, sl])
                    nc.gpsimd.tensor_mul(rot[:], rot[:], sin_sb[:, sl])
                    nc.vector.tensor_add(dest[:], tmp[:], rot[:])

        # -------- Phase 2: attention interleaved with output proj --------
        persist2 = top.enter_context(tc.tile_pool(name="persist2", bufs=1))
        attnT = [[persist2.tile([DH, BLK], bf16, tag=f"attnT{h}_{n}",
                                name=f"attnT{h}_{n}") for n in range(NB)]
                 for h in range(G)]
        wo_sb = persist2.tile([128, G, D], bf16, tag="wo", name="wo_sb")
        wo_r = wo.rearrange("(c p) n -> p c n", p=128)
        for g in range(8):
            nc.gpsimd.dma_start(wo_sb[:, :, g * BLK:(g + 1) * BLK],
                                wo_r[:, :, g * BLK:(g + 1) * BLK])
        with ExitStack() as ph2:
            c2 = ph2.enter_context(tc.tile_pool(name="p2const", bufs=1))
            ones_b = c2.tile([128, 1], bf16, tag="ones_b")
            nc.vector.memset(ones_b[:], 1.0)
            mk_sb = None
            if total_w:
                mk_sb = c2.tile([128, total_w], bf16, tag="mk")
                nc.scalar.dma_start(mk_sb[:], maskm[:])
            ttp = ph2.enter_context(tc.tile_pool(name="ttp", bufs=2))
            ttsp = ph2.enter_context(tc.tile_pool(name="ttsp", bufs=2))
            wtp = ph2.enter_context(tc.tile_pool(name="wtp", bufs=3))
            wtsp = ph2.enter_context(tc.tile_pool(name="wtsp", bufs=3))
            wsp = ph2.enter_context(tc.tile_pool(name="wsp", bufs=2))
            dsp = ph2.enter_context(tc.tile_pool(name="dsp", bufs=2))
            bcp = ph2.enter_context(tc.tile_pool(name="bcp", bufs=2))
            osb = ph2.enter_context(tc.tile_pool(name="osb", bufs=4))
            qkps = ph2.enter_context(
                tc.tile_pool(name="qkps", bufs=3, space="PSUM"))
            avps = ph2.enter_context(
                tc.tile_pool(name="avps", bufs=2, space="PSUM"))
            wops = ph2.enter_context(
                tc.tile_pool(name="wops", bufs=2, space="PSUM"))
            dnps = ph2.enter_context(
                tc.tile_pool(name="dnps", bufs=1, space="PSUM"))

            # wo output-projection work is queued in (row-slice, out-col)
            # units and drained one unit at a time between attention chunks,
            # so the static schedule interleaves these PE-dense matmuls into
            # every scalar-engine wait slot.
            wo_work = []

            def wo_unit(s, nn):
                n2, j = divmod(s, BLK // 128)
                pso = wops.tile([128, BLK], fp32, tag="wop", name="wop")
                for h2 in range(G):
                    nc.tensor.matmul(
                        pso[:], attnT[h2][n2][:, j * 128:(j + 1) * 128],
                        wo_sb[:, h2, nn * BLK:(nn + 1) * BLK],
                        start=(h2 == 0), stop=(h2 == G - 1),
                        skip_group_check=True)
                ot = osb.tile([128, BLK], fp32, tag="ot", name="ot")
                nc.vector.tensor_copy(ot[:], pso[:])
                nc.sync.dma_start(
                    out[s * 128:(s + 1) * 128,
                        nn * BLK:(nn + 1) * BLK], ot[:])

            def pop_wo(k):
                while k > 0 and wo_work:
                    wo_unit(*wo_work.pop(0))
                    k -= 1

            for n in range(NB):
                acts = active[n]
                plains = [c for (c, lo, pe, off) in acts if pe == 0]
                diags = [(c, lo, pe, off) for (c, lo, pe, off) in acts
                         if pe != 0]
                assert len(plains) % 2 == 0 and len(diags) <= 4
                pairs = [(plains[i], plains[i + 1])
                         for i in range(0, len(plains), 2)]
                n_ch = len(acts)
                for h in range(G):
                    av = avps.tile([128, BLK], fp32, tag="av")
                    ws = wsp.tile([128, 2 * BLK], bf16, tag="ws")
                    first_par = [True, True]
                    vstart = [0, 0]
                    mm_i = 0
                    for (c0, c1) in pairs:
                        tt = ttp.tile([128, 2 * BLK], fp32, tag="tt")
                        for i, c in enumerate((c0, c1)):
                            qk = qkps.tile([128, BLK], fp32, tag="qk")
                            nc.tensor.matmul(
                                qk[:],
                                kT[c // 4][:, (c % 4) * 128:(c % 4 + 1) * 128],
                                qT[h][n][:], start=True, stop=True)
                            nc.scalar.activation(
                                tt[:, i * BLK:(i + 1) * BLK], qk[:],
                                AF.Tanh, scale=1.0 / SOFTCAP)
                        wt = wtp.tile([128, 2 * BLK], bf16, tag="wt")
                        nc.scalar.activation(wt[:], tt[:], AF.Exp,
                                             scale=SOFTCAP)
                        assert c0 % 2 == 0 and c1 == c0 + 1
                        if first_par[0]:
                            nc.vector.tensor_copy(ws[:], wt[:])
                            first_par = [False, False]
                        else:
                            nc.vector.tensor_add(ws[:], ws[:], wt[:])
                        for i, c in enumerate((c0, c1)):
                            nc.tensor.matmul(av[:],
                                             vnat[c // 4][:, (c % 4) * 128:
                                                          (c % 4 + 1) * 128],
                                             wt[:, i * BLK:(i + 1) * BLK],
                                             start=(mm_i == 0),
                                             stop=(mm_i == n_ch - 1),
                                             skip_group_check=True)
                            mm_i += 1
                            pop_wo(1)
                    # Diagonal chunks: sub-ranged per-chunk pipeline so each
                    # chunk's AV matmul releases as soon as its exp lands.
                    for (c, lo, pe, off) in diags:
                        qk = qkps.tile([128, BLK], fp32, tag="qk")
                        nc.tensor.matmul(
                            qk[:, lo:],
                            kT[c // 4][:, (c % 4) * 128:(c % 4 + 1) * 128],
                            qT[h][n][:, lo:], start=True, stop=True)
                        tts_t = ttsp.tile([128, BLK], fp32, tag="tts")
                        nc.scalar.activation(tts_t[:, lo:], qk[:, lo:],
                                             AF.Tanh, scale=1.0 / SOFTCAP)
                        wts_t = wtsp.tile([128, BLK], bf16, tag="wts")
                        nc.scalar.activation(wts_t[:, lo:], tts_t[:, lo:],
                                             AF.Exp, scale=SOFTCAP)
                        nc.vector.tensor_mul(wts_t[:, lo:pe],
                                             wts_t[:, lo:pe],
                                             mk_sb[:, off:off + pe - lo])
                        p = c % 2
                        dst = ws[:, p * BLK + lo:(p + 1) * BLK]
                        if first_par[p]:
                            nc.vector.tensor_copy(dst, wts_t[:, lo:])
                            first_par[p] = False
                            vstart[p] = lo
                        else:
                            nc.vector.tensor_add(dst, dst, wts_t[:, lo:])
                        nc.tensor.matmul(av[:, lo:],
                                         vnat[c // 4][:, (c % 4) * 128:
                                                      (c % 4 + 1) * 128],
                                         wts_t[:, lo:],
                                         start=(mm_i == 0),
                                         stop=(mm_i == n_ch - 1),
                                         skip_group_check=True)
                        mm_i += 1
                        pop_wo(1)
                    # denominator: ones-matmul over both parity halves
                    dn = dnps.tile([1, BLK], fp32, tag="dn")
                    for p in (0, 1):
                        vs = vstart[p]
                        nc.tensor.matmul(dn[:, vs:], ones_b[:],
                                         ws[:, p * BLK + vs:(p + 1) * BLK],
                                         start=(p == 0), stop=(p == 1),
                                         skip_group_check=True)
                    dns = dsp.tile([1, BLK], fp32, tag="dns")
                    nc.vector.reciprocal_approx_fast(dns[:], dn[:])
                    bc = bcp.tile([128, BLK], fp32, tag="bc")
                    nc.gpsimd.partition_broadcast(bc[:], dns[:])
                    nc.vector.tensor_mul(attnT[h][n][:], av[:], bc[:])
                    pop_wo(2)
                wo_work.extend((n * (BLK // 128) + j, nn)
                               for j in range(BLK // 128)
                               for nn in range(D // BLK))
            pop_wo(len(wo_work))

    nc.compile()
    return nc


def _rope_tables():
    j = np.arange(0, DH, 2, dtype=np.float32)
    inv = np.float32(1.0) / (np.float32(ROPE_BASE) ** (j / np.float32(DH)))
    t = np.arange(S, dtype=np.float32)
    phase = t[:, None] * inv[None, :]          # [S, 64] fp32 like reference
    cos = np.cos(phase).astype(np.float32)     # [S, 64]
    sin = np.sin(phase).astype(np.float32)
    cosT = np.concatenate([cos.T, cos.T], axis=0)              # [128, S]
    sinT = np.concatenate([-sin.T, sin.T], axis=0)             # sign-folded
    return np.ascontiguousarray(cosT), np.ascontiguousarray(sinT)


def _in_maps(hidden_states, mask, Wq, Wk, Wv, Wo):
    import ml_dtypes
    bf16 = ml_dtypes.bfloat16

    hs = np.asarray(hidden_states, dtype=np.float32).reshape(S, D)
    Wq = np.asarray(Wq, dtype=np.float32)
    Wk = np.asarray(Wk, dtype=np.float32)
    Wv = np.asarray(Wv, dtype=np.float32)
    Wo = np.asarray(Wo, dtype=np.float32)
    active, mtiles = _classify_mask(mask)
    mt = (np.concatenate(mtiles, axis=1).astype(bf16)
          if mtiles else None)
    hsT = np.ascontiguousarray(hs.T)
    cosT, sinT = _rope_tables()
    maps = []
    for c in range(N_CORES):
        m = {
            "hsT": hsT,
            "wq": np.ascontiguousarray(
                Wq[:, c * G * DH:(c + 1) * G * DH] * np.float32(MULT)),
            "wk": np.ascontiguousarray(Wk[:, c * DH:(c + 1) * DH]),
            "wv": np.ascontiguousarray(Wv[:, c * DH:(c + 1) * DH]),
            "wo": np.ascontiguousarray(
                Wo[c * G * DH:(c + 1) * G * DH, :]).astype(bf16),
            "cosT": cosT,
            "sinT": sinT,
        }
        if mt is not None:
            m["maskm"] = np.ascontiguousarray(mt)
        maps.append(m)
    return active, mt, maps


def kernel(hidden_states, mask, Wq, Wk, Wv, Wo):
    from concourse.bass_utils import run_bass_kernel_spmd

    active, mt, maps = _in_maps(hidden_states, mask, Wq, Wk, Wv, Wo)
    key = active
    if key not in _CACHE:
        _CACHE[key] = _build(active, 0 if mt is None else mt.shape[1])
    nc = _CACHE[key]

    res = run_bass_kernel_spmd(nc, maps, list(range(N_CORES)))
    acc = np.zeros((S, D), dtype=np.float64)
    for c in range(N_CORES):
        acc += res.results[c]["out"]
    return acc.astype(np.float32).reshape(1, S, D)


# revision 25
# speedup vs baseline: 1.0568x; 1.0568x over previous
"""GQA multi-head attention (RoPE + tanh softcap + causal mask) on 8 TRN2 cores.

Sharding: tensor-parallel over the 8 kv-head groups (1 kv head + its 4 q heads
per core).  Each core computes its Q/K/V projections from the full hidden
states, runs attention for its 4 q heads, and produces a partial output
through its row-slice of Wo; the host sums the 8 partials.

v2 layout/engine strategy (per core):
  - Q/K path stays float32r end-to-end (logits accurate to ~1e-4); the V/Wo
    path runs in bf16 (V, attention weights post-exp, attnT, Wo) which is
    insensitive at the 2e-2 tolerance and halves SBUF/DMA on that side.
  - logits are computed transposed ([kcol, qrow]); tanh softcap bounds them
    to +-30 so softmax needs no running max.  Diagonal (causally mixed)
    128-col chunks are computed on the visible qrow sub-range only, and the
    {0,1} mask multiply shrinks to the 128-wide partial strip.
  - softmax denominator: bf16 row-sums accumulated on the vector engine,
    reduced across partitions with a ones-matmul, inverted with the fast
    custom-DVE reciprocal (~5x faster than InstReciprocal), broadcast via
    gpsimd, applied on the vector engine.
  - DMA queues are specialized: hidden states stream on the sync queue
    (HWDGE) alone; RoPE rotate-half copies and mask tiles go on the scalar
    queue; weights load on gpsimd/scalar/vector in d-chunk order so the
    first projection matmuls start within a few us.
  - Wo PSUM evictions alternate vector/scalar engines to balance load.
"""

import numpy as np

S, D, DH = 2048, 4096, 128
HQ, HKV = 32, 8
G = HQ // HKV            # q heads per core
N_CORES = 8
MULT = 0.08838834764831845
SOFTCAP = 30.0
ROPE_BASE = 10000.0
BLK = 512                # seq block (matmul moving-dim max for 4-byte dtypes)
NB = S // BLK            # 4 seq blocks
NCH = S // 128           # 16 kcol chunks
NDC = D // 128           # 32 contraction chunks for projections

_CACHE = {}


def _classify_mask(mask):
    """Per (qblock, kchunk) in the transposed [kcol, qrow-local] layout:
    skip (all masked), plain (all visible), or mixed.  Mixed chunks carry
    (lo, pe, off): visible qrow cols form the suffix [lo, 512); cols in
    [lo, pe) are partially masked (mask tile at offset `off` in the packed
    [128, total_width] bf16 mask tensor); cols [pe, 512) are fully visible.
    """
    m = np.asarray(mask).reshape(S, S)
    active = []
    mtiles = []
    off = 0
    for n in range(NB):
        rows = m[n * BLK:(n + 1) * BLK]
        lst = []
        for c in range(NCH):
            sub = rows[:, c * 128:(c + 1) * 128]   # [qrow-local, kcol]
            vis = sub.any(axis=1)
            if not vis.any():
                continue
            full = sub.all(axis=1)
            if full.all():
                lst.append((c, 0, 0, -1))
                continue
            lo = int(np.argmax(vis))
            # visibility must be a suffix, and full-visibility a suffix of it
            assert vis[lo:].all(), "mask rows must be a suffix per chunk"
            if full[lo:].any():
                pe = lo + int(np.argmax(full[lo:]))
                assert full[pe:].all(), "full rows must form a suffix"
            else:
                pe = BLK
            lst.append((c, lo, pe, off))
            mtiles.append(np.ascontiguousarray(sub[lo:pe, :].T))  # [128, pe-lo]
            off += pe - lo
        assert lst and lst[0][0] == 0 and lst[0][1] == 0, \
            "first active chunk must cover qrow col 0"
        active.append(tuple(lst))
    return tuple(active), mtiles


def _build(active, total_w):
    import concourse.bacc as bacc
    import concourse.mybir as mybir
    from concourse import tile
    from concourse.masks import make_identity
    from contextlib import ExitStack

    fp32 = mybir.dt.float32
    f32r = mybir.dt.float32r
    bf16 = mybir.dt.bfloat16
    AF = mybir.ActivationFunctionType

    nc = bacc.Bacc("TRN2", target_bir_lowering=False, debug=False,
                   enable_asserts=True, num_devices=N_CORES)
    hsT = nc.dram_tensor("hsT", [D, S], f32r, kind="ExternalInput").ap()
    wq = nc.dram_tensor("wq", [D, G * DH], f32r, kind="ExternalInput").ap()
    wk = nc.dram_tensor("wk", [D, DH], f32r, kind="ExternalInput").ap()
    wv = nc.dram_tensor("wv", [D, DH], f32r, kind="ExternalInput").ap()
    wo = nc.dram_tensor("wo", [G * DH, D], bf16, kind="ExternalInput").ap()
    cosT = nc.dram_tensor("cosT", [DH, S], fp32, kind="ExternalInput").ap()
    sinT = nc.dram_tensor("sinT", [DH, S], fp32, kind="ExternalInput").ap()
    maskm = (nc.dram_tensor("maskm", [128, total_w], bf16,
                            kind="ExternalInput").ap() if total_w else None)
    out = nc.dram_tensor("out", [S, D], fp32, kind="ExternalOutput").ap()

    with tile.TileContext(nc) as tc, ExitStack() as top:
        persist = top.enter_context(tc.tile_pool(name="persist", bufs=1))
        qT = [[persist.tile([DH, BLK], f32r, tag=f"qT{h}_{n}",
                            name=f"qT{h}_{n}") for n in range(NB)]
              for h in range(G)]
        kT = [persist.tile([DH, BLK], f32r, tag=f"kT{n}", name=f"kT{n}")
              for n in range(NB)]
        vnat = [persist.tile([128, BLK], bf16, tag=f"vnat{n}",
                             name=f"vnat{n}") for n in range(NB)]

        # ---------------- Phase 1: QKV projections + RoPE ----------------
        with ExitStack() as ph1:
            const = ph1.enter_context(tc.tile_pool(name="p1const", bufs=1))
            wq_sb = const.tile([128, NDC, G * DH], f32r, tag="wq")
            wk_sb = const.tile([128, NDC, DH], f32r, tag="wk")
            wv_sb = const.tile([128, NDC, DH], f32r, tag="wv")
            cos_sb = const.tile([DH, S], fp32, tag="cos")
            sin_sb = const.tile([DH, S], fp32, tag="sin")
            identb = const.tile([128, 128], bf16, tag="identb")
            wq_r = wq.rearrange("(c p) m -> p c m", p=128)
            wk_r = wk.rearrange("(c p) m -> p c m", p=128)
            wv_r = wv.rearrange("(c p) m -> p c m", p=128)
            # d-chunk-ordered weight loads on three queues so the d=0
            # matmuls can start within a few us.
            for g in range(8):
                gs = slice(g * 4, (g + 1) * 4)
                nc.scalar.dma_start(wq_sb[:, gs, :], wq_r[:, gs, :])
                nc.scalar.dma_start(wk_sb[:, gs, :], wk_r[:, gs, :])
                nc.scalar.dma_start(wv_sb[:, gs, :], wv_r[:, gs, :])
            nc.gpsimd.dma_start(cos_sb[:], cosT[:])
            nc.gpsimd.dma_start(sin_sb[:], sinT[:])
            make_identity(nc, identb[:])

            hsp = ph1.enter_context(tc.tile_pool(name="hs", bufs=10))
            pps = ph1.enter_context(
                tc.tile_pool(name="projps", bufs=7, space="PSUM"))
            rawp = ph1.enter_context(tc.tile_pool(name="raw", bufs=3))
            rotp = ph1.enter_context(tc.tile_pool(name="rot", bufs=3))
            tmpp = ph1.enter_context(tc.tile_pool(name="tmp", bufs=3))
            vtp = ph1.enter_context(tc.tile_pool(name="vtp", bufs=2))
            tps = ph1.enter_context(
                tc.tile_pool(name="tps", bufs=1, space="PSUM"))

            for n in range(NB):
                sl = slice(n * BLK, (n + 1) * BLK)
                ps = [pps.tile([128, BLK], fp32, tag="projps", name="projps")
                      for _ in range(G + 2)]
                for d in range(NDC):
                    hs_t = hsp.tile([128, BLK], f32r, tag="hs")
                    nc.sync.dma_start(hs_t[:], hsT[d * 128:(d + 1) * 128, sl])
                    for h in range(G):
                        nc.tensor.matmul(ps[h][:],
                                         wq_sb[:, d, h * DH:(h + 1) * DH],
                                         hs_t[:], start=(d == 0),
                                         stop=(d == NDC - 1))
                    nc.tensor.matmul(ps[G][:], wk_sb[:, d, :], hs_t[:],
                                     start=(d == 0), stop=(d == NDC - 1))
                    nc.tensor.matmul(ps[G + 1][:], wv_sb[:, d, :], hs_t[:],
                                     start=(d == 0), stop=(d == NDC - 1))
                # V first: evict to bf16, PE-transpose 128-chunks into one
                # PSUM bank, evict once to vnat[n] ([kcol, dh] per chunk).
                # V-first keeps the PE busy with transposes and frees its
                # bank before the scalar/vector RoPE chain runs.
                vt = vtp.tile([128, BLK], bf16, tag="vt")
                nc.scalar.copy(vt[:], ps[G + 1][:])
                tp = tps.tile([128, BLK], bf16, tag="tp")
                for j in range(BLK // 128):
                    nc.tensor.matmul(tp[:, j * 128:(j + 1) * 128],
                                     vt[:, j * 128:(j + 1) * 128], identb[:],
                                     is_transpose=True, start=True, stop=True,
                                     skip_group_check=True)
                nc.vector.tensor_copy(vnat[n][:], tp[:])
                # RoPE on q heads and k: evict PSUM, then rotate-half via
                # scalar-queue SBUF-SBUF DMA (keeps the sync queue for hs).
                for i, dest in enumerate([qT[h][n] for h in range(G)]
                                         + [kT[n]]):
                    raw = rawp.tile([128, BLK], fp32, tag="raw")
                    if i % 2 == 0:
                        nc.scalar.copy(raw[:], ps[i][:])
                    else:
                        nc.vector.tensor_copy(raw[:], ps[i][:])
                    rot = rotp.tile([128, BLK], fp32, tag="rot")
                    nc.scalar.dma_start(rot[0:64, :], raw[64:128, :])
                    nc.scalar.dma_start(rot[64:128, :], raw[0:64, :])
                    tmp = tmpp.tile([128, BLK], fp32, tag="tmp")
                    nc.vector.tensor_mul(tmp[:], raw[:], cos_sb[:, sl])
                    nc.gpsimd.tensor_mul(rot[:], rot[:], sin_sb[:, sl])
                    nc.vector.tensor_add(dest[:], tmp[:], rot[:])

        # -------- Phase 2: attention interleaved with output proj --------
        persist2 = top.enter_context(tc.tile_pool(name="persist2", bufs=1))
        attnT = [[persist2.tile([DH, BLK], bf16, tag=f"attnT{h}_{n}",
                                name=f"attnT{h}_{n}") for n in range(NB)]
                 for h in range(G)]
        wo_sb = persist2.tile([128, G, D], bf16, tag="wo", name="wo_sb")
        wo_r = wo.rearrange("(c p) n -> p c n", p=128)
        for g in range(8):
            nc.gpsimd.dma_start(wo_sb[:, :, g * BLK:(g + 1) * BLK],
                                wo_r[:, :, g * BLK:(g + 1) * BLK])
        with ExitStack() as ph2:
            c2 = ph2.enter_context(tc.tile_pool(name="p2const", bufs=1))
            ones_b = c2.tile([128, 1], bf16, tag="ones_b")
            nc.vector.memset(ones_b[:], 1.0)
            mk_sb = None
            if total_w:
                mk_sb = c2.tile([128, total_w], bf16, tag="mk")
                nc.scalar.dma_start(mk_sb[:], maskm[:])
            ttp = ph2.enter_context(tc.tile_pool(name="ttp", bufs=2))
            ttsp = ph2.enter_context(tc.tile_pool(name="ttsp", bufs=2))
            wtp = ph2.enter_context(tc.tile_pool(name="wtp", bufs=3))
            wtsp = ph2.enter_context(tc.tile_pool(name="wtsp", bufs=3))
            wsp = ph2.enter_context(tc.tile_pool(name="wsp", bufs=2))
            dsp = ph2.enter_context(tc.tile_pool(name="dsp", bufs=2))
            bcp = ph2.enter_context(tc.tile_pool(name="bcp", bufs=2))
            osb = ph2.enter_context(tc.tile_pool(name="osb", bufs=4))
            qkps = ph2.enter_context(
                tc.tile_pool(name="qkps", bufs=3, space="PSUM"))
            avps = ph2.enter_context(
                tc.tile_pool(name="avps", bufs=2, space="PSUM"))
            wops = ph2.enter_context(
                tc.tile_pool(name="wops", bufs=2, space="PSUM"))
            dnps = ph2.enter_context(
                tc.tile_pool(name="dnps", bufs=1, space="PSUM"))

            # wo slices for block n are deferred into block n+1's head loop
            # (one slice per head): the PE-dense wo matmuls then interleave
            # with the scalar-engine-paced softmax stretches instead of
            # running as a separate block-sized burst.
            def wo_slice(s):
                n2, j = divmod(s, BLK // 128)
                for nn in range(D // BLK):
                    pso = wops.tile([128, BLK], fp32, tag="wop", name="wop")
                    for h2 in range(G):
                        nc.tensor.matmul(
                            pso[:], attnT[h2][n2][:, j * 128:(j + 1) * 128],
                            wo_sb[:, h2, nn * BLK:(nn + 1) * BLK],
                            start=(h2 == 0), stop=(h2 == G - 1),
                            skip_group_check=True)
                    ot = osb.tile([128, BLK], fp32, tag="ot", name="ot")
                    nc.vector.tensor_copy(ot[:], pso[:])
                    nc.sync.dma_start(
                        out[s * 128:(s + 1) * 128,
                            nn * BLK:(nn + 1) * BLK], ot[:])

            prev_wo = None
            for n in range(NB):
                acts = active[n]
                plains = [c for (c, lo, pe, off) in acts if pe == 0]
                diags = [(c, lo, pe, off) for (c, lo, pe, off) in acts
                         if pe != 0]
                assert len(plains) % 2 == 0 and len(diags) <= 4
                pairs = [(plains[i], plains[i + 1])
                         for i in range(0, len(plains), 2)]
                n_ch = len(acts)
                for h in range(G):
                    av = avps.tile([128, BLK], fp32, tag="av")
                    ws = wsp.tile([128, 2 * BLK], bf16, tag="ws")
                    first_par = [True, True]
                    vstart = [0, 0]
                    mm_i = 0
                    for (c0, c1) in pairs:
                        tt = ttp.tile([128, 2 * BLK], fp32, tag="tt")
                        for i, c in enumerate((c0, c1)):
                            qk = qkps.tile([128, BLK], fp32, tag="qk")
                            nc.tensor.matmul(
                                qk[:],
                                kT[c // 4][:, (c % 4) * 128:(c % 4 + 1) * 128],
                                qT[h][n][:], start=True, stop=True)
                            nc.scalar.activation(
                                tt[:, i * BLK:(i + 1) * BLK], qk[:],
                                AF.Tanh, scale=1.0 / SOFTCAP)
                        wt = wtp.tile([128, 2 * BLK], bf16, tag="wt")
                        nc.scalar.activation(wt[:], tt[:], AF.Exp,
                                             scale=SOFTCAP)
                        assert c0 % 2 == 0 and c1 == c0 + 1
                        if first_par[0]:
                            nc.vector.tensor_copy(ws[:], wt[:])
                            first_par = [False, False]
                        else:
                            nc.vector.tensor_add(ws[:], ws[:], wt[:])
                        for i, c in enumerate((c0, c1)):
                            nc.tensor.matmul(av[:],
                                             vnat[c // 4][:, (c % 4) * 128:
                                                          (c % 4 + 1) * 128],
                                             wt[:, i * BLK:(i + 1) * BLK],
                                             start=(mm_i == 0),
                                             stop=(mm_i == n_ch - 1),
                                             skip_group_check=True)
                            mm_i += 1
                    # Diagonal chunks: sub-ranged per-chunk pipeline so each
                    # chunk's AV matmul releases as soon as its exp lands.
                    for (c, lo, pe, off) in diags:
                        qk = qkps.tile([128, BLK], fp32, tag="qk")
                        nc.tensor.matmul(
                            qk[:, lo:],
                            kT[c // 4][:, (c % 4) * 128:(c % 4 + 1) * 128],
                            qT[h][n][:, lo:], start=True, stop=True)
                        tts_t = ttsp.tile([128, BLK], fp32, tag="tts")
                        nc.scalar.activation(tts_t[:, lo:], qk[:, lo:],
                                             AF.Tanh, scale=1.0 / SOFTCAP)
                        wts_t = wtsp.tile([128, BLK], bf16, tag="wts")
                        nc.scalar.activation(wts_t[:, lo:], tts_t[:, lo:],
                                             AF.Exp, scale=SOFTCAP)
                        nc.vector.tensor_mul(wts_t[:, lo:pe],
                                             wts_t[:, lo:pe],
                                             mk_sb[:, off:off + pe - lo])
                        p = c % 2
                        dst = ws[:, p * BLK + lo:(p + 1) * BLK]
                        if first_par[p]:
                            nc.vector.tensor_copy(dst, wts_t[:, lo:])
                            first_par[p] = False
                            vstart[p] = lo
                        else:
                            nc.vector.tensor_add(dst, dst, wts_t[:, lo:])
                        nc.tensor.matmul(av[:, lo:],
                                         vnat[c // 4][:, (c % 4) * 128:
                                                      (c % 4 + 1) * 128],
                                         wts_t[:, lo:],
                                         start=(mm_i == 0),
                                         stop=(mm_i == n_ch - 1),
                                         skip_group_check=True)
                        mm_i += 1
                    # denominator: ones-matmul over both parity halves
                    dn = dnps.tile([1, BLK], fp32, tag="dn")
                    for p in (0, 1):
                        vs = vstart[p]
                        nc.tensor.matmul(dn[:, vs:], ones_b[:],
                                         ws[:, p * BLK + vs:(p + 1) * BLK],
                                         start=(p == 0), stop=(p == 1),
                                         skip_group_check=True)
                    dns = dsp.tile([1, BLK], fp32, tag="dns")
                    nc.vector.reciprocal_approx_fast(dns[:], dn[:])
                    bc = bcp.tile([128, BLK], fp32, tag="bc")
                    nc.gpsimd.partition_broadcast(bc[:], dns[:])
                    nc.vector.tensor_mul(attnT[h][n][:], av[:], bc[:])
                    if prev_wo is not None:
                        wo_slice(prev_wo * (BLK // 128) + h)
                prev_wo = n
            for j in range(BLK // 128):
                wo_slice(prev_wo * (BLK // 128) + j)

    nc.compile()
    return nc


def _rope_tables():
    j = np.arange(0, DH, 2, dtype=np.float32)
    inv = np.float32(1.0) / (np.float32(ROPE_BASE) ** (j / np.float32(DH)))
    t = np.arange(S, dtype=np.float32)
    phase = t[:, None] * inv[None, :]          # [S, 64] fp32 like reference
    cos = np.cos(phase).astype(np.float32)     # [S, 64]
    sin = np.sin(phase).astype(np.float32)
    cosT = np.concatenate([cos.T, cos.T], axis=0)              # [128, S]
    sinT = np.concatenate([-sin.T, sin.T], axis=0)             # sign-folded
    return np.ascontiguousarray(cosT), np.ascontiguousarray(sinT)


def _in_maps(hidden_states, mask, Wq, Wk, Wv, Wo):
    import ml_dtypes
    bf16 = ml_dtypes.bfloat16

    hs = np.asarray(hidden_states, dtype=np.float32).reshape(S, D)
    Wq = np.asarray(Wq, dtype=np.float32)
    Wk = np.asarray(Wk, dtype=np.float32)
    Wv = np.asarray(Wv, dtype=np.float32)
    Wo = np.asarray(Wo, dtype=np.float32)
    active, mtiles = _classify_mask(mask)
    mt = (np.concatenate(mtiles, axis=1).astype(bf16)
          if mtiles else None)
    hsT = np.ascontiguousarray(hs.T)
    cosT, sinT = _rope_tables()
    maps = []
    for c in range(N_CORES):
        m = {
            "hsT": hsT,
            "wq": np.ascontiguousarray(
                Wq[:, c * G * DH:(c + 1) * G * DH] * np.float32(MULT)),
            "wk": np.ascontiguousarray(Wk[:, c * DH:(c + 1) * DH]),
            "wv": np.ascontiguousarray(Wv[:, c * DH:(c + 1) * DH]),
            "wo": np.ascontiguousarray(
                Wo[c * G * DH:(c + 1) * G * DH, :]).astype(bf16),
            "cosT": cosT,
            "sinT": sinT,
        }
        if mt is not None:
            m["maskm"] = np.ascontiguousarray(mt)
        maps.append(m)
    return active, mt, maps


def kernel(hidden_states, mask, Wq, Wk, Wv, Wo):
    from concourse.bass_utils import run_bass_kernel_spmd

    active, mt, maps = _in_maps(hidden_states, mask, Wq, Wk, Wv, Wo)
    key = active
    if key not in _CACHE:
        _CACHE[key] = _build(active, 0 if mt is None else mt.shape[1])
    nc = _CACHE[key]

    res = run_bass_kernel_spmd(nc, maps, list(range(N_CORES)))
    acc = np.zeros((S, D), dtype=np.float64)
    for c in range(N_CORES):
        acc += res.results[c]["out"]
    return acc.astype(np.float32).reshape(1, S, D)


# revision 26
# speedup vs baseline: 1.0605x; 1.0035x over previous
"""GQA multi-head attention (RoPE + tanh softcap + causal mask) on 8 TRN2 cores.

Sharding: tensor-parallel over the 8 kv-head groups (1 kv head + its 4 q heads
per core).  Each core computes its Q/K/V projections from the full hidden
states, runs attention for its 4 q heads, and produces a partial output
through its row-slice of Wo; the host sums the 8 partials.

v2 layout/engine strategy (per core):
  - Q/K path stays float32r end-to-end (logits accurate to ~1e-4); the V/Wo
    path runs in bf16 (V, attention weights post-exp, attnT, Wo) which is
    insensitive at the 2e-2 tolerance and halves SBUF/DMA on that side.
  - logits are computed transposed ([kcol, qrow]); tanh softcap bounds them
    to +-30 so softmax needs no running max.  Diagonal (causally mixed)
    128-col chunks are computed on the visible qrow sub-range only, and the
    {0,1} mask multiply shrinks to the 128-wide partial strip.
  - softmax denominator: bf16 row-sums accumulated on the vector engine,
    reduced across partitions with a ones-matmul, inverted with the fast
    custom-DVE reciprocal (~5x faster than InstReciprocal), broadcast via
    gpsimd, applied on the vector engine.
  - DMA queues are specialized: hidden states stream on the sync queue
    (HWDGE) alone; RoPE rotate-half copies and mask tiles go on the scalar
    queue; weights load on gpsimd/scalar/vector in d-chunk order so the
    first projection matmuls start within a few us.
  - Wo PSUM evictions alternate vector/scalar engines to balance load.
"""

import numpy as np

S, D, DH = 2048, 4096, 128
HQ, HKV = 32, 8
G = HQ // HKV            # q heads per core
N_CORES = 8
MULT = 0.08838834764831845
SOFTCAP = 30.0
ROPE_BASE = 10000.0
BLK = 512                # seq block (matmul moving-dim max for 4-byte dtypes)
NB = S // BLK            # 4 seq blocks
NCH = S // 128           # 16 kcol chunks
NDC = D // 128           # 32 contraction chunks for projections

_CACHE = {}


def _classify_mask(mask):
    """Per (qblock, kchunk) in the transposed [kcol, qrow-local] layout:
    skip (all masked), plain (all visible), or mixed.  Mixed chunks carry
    (lo, pe, off): visible qrow cols form the suffix [lo, 512); cols in
    [lo, pe) are partially masked (mask tile at offset `off` in the packed
    [128, total_width] bf16 mask tensor); cols [pe, 512) are fully visible.
    """
    m = np.asarray(mask).reshape(S, S)
    active = []
    mtiles = []
    off = 0
    for n in range(NB):
        rows = m[n * BLK:(n + 1) * BLK]
        lst = []
        for c in range(NCH):
            sub = rows[:, c * 128:(c + 1) * 128]   # [qrow-local, kcol]
            vis = sub.any(axis=1)
            if not vis.any():
                continue
            full = sub.all(axis=1)
            if full.all():
                lst.append((c, 0, 0, -1))
                continue
            lo = int(np.argmax(vis))
            # visibility must be a suffix, and full-visibility a suffix of it
            assert vis[lo:].all(), "mask rows must be a suffix per chunk"
            if full[lo:].any():
                pe = lo + int(np.argmax(full[lo:]))
                assert full[pe:].all(), "full rows must form a suffix"
            else:
                pe = BLK
            lst.append((c, lo, pe, off))
            mtiles.append(np.ascontiguousarray(sub[lo:pe, :].T))  # [128, pe-lo]
            off += pe - lo
        assert lst and lst[0][0] == 0 and lst[0][1] == 0, \
            "first active chunk must cover qrow col 0"
        active.append(tuple(lst))
    return tuple(active), mtiles


def _build(active, total_w):
    import concourse.bacc as bacc
    import concourse.mybir as mybir
    from concourse import tile
    from concourse.masks import make_identity
    from contextlib import ExitStack

    fp32 = mybir.dt.float32
    f32r = mybir.dt.float32r
    bf16 = mybir.dt.bfloat16
    AF = mybir.ActivationFunctionType

    nc = bacc.Bacc("TRN2", target_bir_lowering=False, debug=False,
                   enable_asserts=True, num_devices=N_CORES)
    hsT = nc.dram_tensor("hsT", [D, S], f32r, kind="ExternalInput").ap()
    wq = nc.dram_tensor("wq", [D, G * DH], f32r, kind="ExternalInput").ap()
    wk = nc.dram_tensor("wk", [D, DH], f32r, kind="ExternalInput").ap()
    wv = nc.dram_tensor("wv", [D, DH], f32r, kind="ExternalInput").ap()
    wo = nc.dram_tensor("wo", [G * DH, D], bf16, kind="ExternalInput").ap()
    cosT = nc.dram_tensor("cosT", [DH, S], fp32, kind="ExternalInput").ap()
    sinT = nc.dram_tensor("sinT", [DH, S], fp32, kind="ExternalInput").ap()
    maskm = (nc.dram_tensor("maskm", [128, total_w], bf16,
                            kind="ExternalInput").ap() if total_w else None)
    out = nc.dram_tensor("out", [S, D], fp32, kind="ExternalOutput").ap()

    with tile.TileContext(nc) as tc, ExitStack() as top:
        persist = top.enter_context(tc.tile_pool(name="persist", bufs=1))
        qT = [[persist.tile([DH, BLK], f32r, tag=f"qT{h}_{n}",
                            name=f"qT{h}_{n}") for n in range(NB)]
              for h in range(G)]
        kT = [persist.tile([DH, BLK], f32r, tag=f"kT{n}", name=f"kT{n}")
              for n in range(NB)]
        vnat = [persist.tile([128, BLK], bf16, tag=f"vnat{n}",
                             name=f"vnat{n}") for n in range(NB)]

        # ---------------- Phase 1: QKV projections + RoPE ----------------
        with ExitStack() as ph1:
            const = ph1.enter_context(tc.tile_pool(name="p1const", bufs=1))
            wq_sb = const.tile([128, NDC, G * DH], f32r, tag="wq")
            wk_sb = const.tile([128, NDC, DH], f32r, tag="wk")
            wv_sb = const.tile([128, NDC, DH], f32r, tag="wv")
            cos_sb = const.tile([DH, S], fp32, tag="cos")
            sin_sb = const.tile([DH, S], fp32, tag="sin")
            identb = const.tile([128, 128], bf16, tag="identb")
            wq_r = wq.rearrange("(c p) m -> p c m", p=128)
            wk_r = wk.rearrange("(c p) m -> p c m", p=128)
            wv_r = wv.rearrange("(c p) m -> p c m", p=128)
            # d-chunk-ordered weight loads on three queues so the d=0
            # matmuls can start within a few us.
            # small wk/wv chunks first so the d=0 K/V matmuls aren't stuck
            # behind the 1 MB wq chunk on the scalar queue at startup
            for g in range(8):
                gs = slice(g * 4, (g + 1) * 4)
                nc.scalar.dma_start(wk_sb[:, gs, :], wk_r[:, gs, :])
                nc.scalar.dma_start(wv_sb[:, gs, :], wv_r[:, gs, :])
                nc.scalar.dma_start(wq_sb[:, gs, :], wq_r[:, gs, :])
            nc.gpsimd.dma_start(cos_sb[:], cosT[:])
            nc.gpsimd.dma_start(sin_sb[:], sinT[:])
            make_identity(nc, identb[:])

            hsp = ph1.enter_context(tc.tile_pool(name="hs", bufs=10))
            pps = ph1.enter_context(
                tc.tile_pool(name="projps", bufs=7, space="PSUM"))
            rawp = ph1.enter_context(tc.tile_pool(name="raw", bufs=3))
            rotp = ph1.enter_context(tc.tile_pool(name="rot", bufs=3))
            tmpp = ph1.enter_context(tc.tile_pool(name="tmp", bufs=3))
            vtp = ph1.enter_context(tc.tile_pool(name="vtp", bufs=2))
            tps = ph1.enter_context(
                tc.tile_pool(name="tps", bufs=1, space="PSUM"))

            for n in range(NB):
                sl = slice(n * BLK, (n + 1) * BLK)
                ps = [pps.tile([128, BLK], fp32, tag="projps", name="projps")
                      for _ in range(G + 2)]
                for d in range(NDC):
                    hs_t = hsp.tile([128, BLK], f32r, tag="hs")
                    nc.sync.dma_start(hs_t[:], hsT[d * 128:(d + 1) * 128, sl])
                    for h in range(G):
                        nc.tensor.matmul(ps[h][:],
                                         wq_sb[:, d, h * DH:(h + 1) * DH],
                                         hs_t[:], start=(d == 0),
                                         stop=(d == NDC - 1))
                    nc.tensor.matmul(ps[G][:], wk_sb[:, d, :], hs_t[:],
                                     start=(d == 0), stop=(d == NDC - 1))
                    nc.tensor.matmul(ps[G + 1][:], wv_sb[:, d, :], hs_t[:],
                                     start=(d == 0), stop=(d == NDC - 1))
                # V first: evict to bf16, PE-transpose 128-chunks into one
                # PSUM bank, evict once to vnat[n] ([kcol, dh] per chunk).
                # V-first keeps the PE busy with transposes and frees its
                # bank before the scalar/vector RoPE chain runs.
                vt = vtp.tile([128, BLK], bf16, tag="vt")
                nc.scalar.copy(vt[:], ps[G + 1][:])
                tp = tps.tile([128, BLK], bf16, tag="tp")
                for j in range(BLK // 128):
                    nc.tensor.matmul(tp[:, j * 128:(j + 1) * 128],
                                     vt[:, j * 128:(j + 1) * 128], identb[:],
                                     is_transpose=True, start=True, stop=True,
                                     skip_group_check=True)
                nc.vector.tensor_copy(vnat[n][:], tp[:])
                # RoPE on q heads and k: evict PSUM, then rotate-half via
                # scalar-queue SBUF-SBUF DMA (keeps the sync queue for hs).
                for i, dest in enumerate([qT[h][n] for h in range(G)]
                                         + [kT[n]]):
                    raw = rawp.tile([128, BLK], fp32, tag="raw")
                    if i % 2 == 0:
                        nc.scalar.copy(raw[:], ps[i][:])
                    else:
                        nc.vector.tensor_copy(raw[:], ps[i][:])
                    rot = rotp.tile([128, BLK], fp32, tag="rot")
                    nc.scalar.dma_start(rot[0:64, :], raw[64:128, :])
                    nc.scalar.dma_start(rot[64:128, :], raw[0:64, :])
                    tmp = tmpp.tile([128, BLK], fp32, tag="tmp")
                    nc.vector.tensor_mul(tmp[:], raw[:], cos_sb[:, sl])
                    nc.gpsimd.tensor_mul(rot[:], rot[:], sin_sb[:, sl])
                    nc.vector.tensor_add(dest[:], tmp[:], rot[:])

        # -------- Phase 2: attention interleaved with output proj --------
        persist2 = top.enter_context(tc.tile_pool(name="persist2", bufs=1))
        attnT = [[persist2.tile([DH, BLK], bf16, tag=f"attnT{h}_{n}",
                                name=f"attnT{h}_{n}") for n in range(NB)]
                 for h in range(G)]
        wo_sb = persist2.tile([128, G, D], bf16, tag="wo", name="wo_sb")
        wo_r = wo.rearrange("(c p) n -> p c n", p=128)
        for g in range(8):
            nc.gpsimd.dma_start(wo_sb[:, :, g * BLK:(g + 1) * BLK],
                                wo_r[:, :, g * BLK:(g + 1) * BLK])
        with ExitStack() as ph2:
            c2 = ph2.enter_context(tc.tile_pool(name="p2const", bufs=1))
            ones_b = c2.tile([128, 1], bf16, tag="ones_b")
            nc.vector.memset(ones_b[:], 1.0)
            mk_sb = None
            if total_w:
                mk_sb = c2.tile([128, total_w], bf16, tag="mk")
                nc.scalar.dma_start(mk_sb[:], maskm[:])
            ttp = ph2.enter_context(tc.tile_pool(name="ttp", bufs=2))
            ttsp = ph2.enter_context(tc.tile_pool(name="ttsp", bufs=2))
            wtp = ph2.enter_context(tc.tile_pool(name="wtp", bufs=3))
            wtsp = ph2.enter_context(tc.tile_pool(name="wtsp", bufs=3))
            wsp = ph2.enter_context(tc.tile_pool(name="wsp", bufs=2))
            dsp = ph2.enter_context(tc.tile_pool(name="dsp", bufs=2))
            bcp = ph2.enter_context(tc.tile_pool(name="bcp", bufs=2))
            osb = ph2.enter_context(tc.tile_pool(name="osb", bufs=4))
            qkps = ph2.enter_context(
                tc.tile_pool(name="qkps", bufs=3, space="PSUM"))
            avps = ph2.enter_context(
                tc.tile_pool(name="avps", bufs=2, space="PSUM"))
            wops = ph2.enter_context(
                tc.tile_pool(name="wops", bufs=2, space="PSUM"))
            dnps = ph2.enter_context(
                tc.tile_pool(name="dnps", bufs=1, space="PSUM"))

            # wo slices for block n are deferred into block n+1's head loop
            # (one slice per head): the PE-dense wo matmuls then interleave
            # with the scalar-engine-paced softmax stretches instead of
            # running as a separate block-sized burst.
            def wo_slice(s):
                n2, j = divmod(s, BLK // 128)
                for nn in range(D // BLK):
                    pso = wops.tile([128, BLK], fp32, tag="wop", name="wop")
                    for h2 in range(G):
                        nc.tensor.matmul(
                            pso[:], attnT[h2][n2][:, j * 128:(j + 1) * 128],
                            wo_sb[:, h2, nn * BLK:(nn + 1) * BLK],
                            start=(h2 == 0), stop=(h2 == G - 1),
                            skip_group_check=True)
                    ot = osb.tile([128, BLK], fp32, tag="ot", name="ot")
                    nc.vector.tensor_copy(ot[:], pso[:])
                    nc.sync.dma_start(
                        out[s * 128:(s + 1) * 128,
                            nn * BLK:(nn + 1) * BLK], ot[:])

            prev_wo = None
            for n in range(NB):
                acts = active[n]
                plains = [c for (c, lo, pe, off) in acts if pe == 0]
                diags = [(c, lo, pe, off) for (c, lo, pe, off) in acts
                         if pe != 0]
                assert len(plains) % 2 == 0 and len(diags) <= 4
                pairs = [(plains[i], plains[i + 1])
                         for i in range(0, len(plains), 2)]
                n_ch = len(acts)
                for h in range(G):
                    av = avps.tile([128, BLK], fp32, tag="av")
                    ws = wsp.tile([128, 2 * BLK], bf16, tag="ws")
                    first_par = [True, True]
                    vstart = [0, 0]
                    mm_i = 0
                    for (c0, c1) in pairs:
                        tt = ttp.tile([128, 2 * BLK], fp32, tag="tt")
                        for i, c in enumerate((c0, c1)):
                            qk = qkps.tile([128, BLK], fp32, tag="qk")
                            nc.tensor.matmul(
                                qk[:],
                                kT[c // 4][:, (c % 4) * 128:(c % 4 + 1) * 128],
                                qT[h][n][:], start=True, stop=True)
                            nc.scalar.activation(
                                tt[:, i * BLK:(i + 1) * BLK], qk[:],
                                AF.Tanh, scale=1.0 / SOFTCAP)
                        wt = wtp.tile([128, 2 * BLK], bf16, tag="wt")
                        nc.scalar.activation(wt[:], tt[:], AF.Exp,
                                             scale=SOFTCAP)
                        assert c0 % 2 == 0 and c1 == c0 + 1
                        if first_par[0]:
                            nc.vector.tensor_copy(ws[:], wt[:])
                            first_par = [False, False]
                        else:
                            nc.vector.tensor_add(ws[:], ws[:], wt[:])
                        for i, c in enumerate((c0, c1)):
                            nc.tensor.matmul(av[:],
                                             vnat[c // 4][:, (c % 4) * 128:
                                                          (c % 4 + 1) * 128],
                                             wt[:, i * BLK:(i + 1) * BLK],
                                             start=(mm_i == 0),
                                             stop=(mm_i == n_ch - 1),
                                             skip_group_check=True)
                            mm_i += 1
                    # Diagonal chunks: sub-ranged per-chunk pipeline so each
                    # chunk's AV matmul releases as soon as its exp lands.
                    for (c, lo, pe, off) in diags:
                        qk = qkps.tile([128, BLK], fp32, tag="qk")
                        nc.tensor.matmul(
                            qk[:, lo:],
                            kT[c // 4][:, (c % 4) * 128:(c % 4 + 1) * 128],
                            qT[h][n][:, lo:], start=True, stop=True)
                        tts_t = ttsp.tile([128, BLK], fp32, tag="tts")
                        nc.scalar.activation(tts_t[:, lo:], qk[:, lo:],
                                             AF.Tanh, scale=1.0 / SOFTCAP)
                        wts_t = wtsp.tile([128, BLK], bf16, tag="wts")
                        nc.scalar.activation(wts_t[:, lo:], tts_t[:, lo:],
                                             AF.Exp, scale=SOFTCAP)
                        nc.vector.tensor_mul(wts_t[:, lo:pe],
                                             wts_t[:, lo:pe],
                                             mk_sb[:, off:off + pe - lo])
                        p = c % 2
                        dst = ws[:, p * BLK + lo:(p + 1) * BLK]
                        if first_par[p]:
                            nc.vector.tensor_copy(dst, wts_t[:, lo:])
                            first_par[p] = False
                            vstart[p] = lo
                        else:
                            nc.vector.tensor_add(dst, dst, wts_t[:, lo:])
                        nc.tensor.matmul(av[:, lo:],
                                         vnat[c // 4][:, (c % 4) * 128:
                                                      (c % 4 + 1) * 128],
                                         wts_t[:, lo:],
                                         start=(mm_i == 0),
                                         stop=(mm_i == n_ch - 1),
                                         skip_group_check=True)
                        mm_i += 1
                    # denominator: ones-matmul over both parity halves
                    dn = dnps.tile([1, BLK], fp32, tag="dn")
                    for p in (0, 1):
                        vs = vstart[p]
                        nc.tensor.matmul(dn[:, vs:], ones_b[:],
                                         ws[:, p * BLK + vs:(p + 1) * BLK],
                                         start=(p == 0), stop=(p == 1),
                                         skip_group_check=True)
                    dns = dsp.tile([1, BLK], fp32, tag="dns")
                    nc.vector.reciprocal_approx_fast(dns[:], dn[:])
                    bc = bcp.tile([128, BLK], fp32, tag="bc")
                    nc.gpsimd.partition_broadcast(bc[:], dns[:])
                    nc.vector.tensor_mul(attnT[h][n][:], av[:], bc[:])
                    if prev_wo is not None:
                        wo_slice(prev_wo * (BLK // 128) + h)
                prev_wo = n
            for j in range(BLK // 128):
                wo_slice(prev_wo * (BLK // 128) + j)

    nc.compile()
    return nc


def _rope_tables():
    j = np.arange(0, DH, 2, dtype=np.float32)
    inv = np.float32(1.0) / (np.float32(ROPE_BASE) ** (j / np.float32(DH)))
    t = np.arange(S, dtype=np.float32)
    phase = t[:, None] * inv[None, :]          # [S, 64] fp32 like reference
    cos = np.cos(phase).astype(np.float32)     # [S, 64]
    sin = np.sin(phase).astype(np.float32)
    cosT = np.concatenate([cos.T, cos.T], axis=0)              # [128, S]
    sinT = np.concatenate([-sin.T, sin.T], axis=0)             # sign-folded
    return np.ascontiguousarray(cosT), np.ascontiguousarray(sinT)


def _in_maps(hidden_states, mask, Wq, Wk, Wv, Wo):
    import ml_dtypes
    bf16 = ml_dtypes.bfloat16

    hs = np.asarray(hidden_states, dtype=np.float32).reshape(S, D)
    Wq = np.asarray(Wq, dtype=np.float32)
    Wk = np.asarray(Wk, dtype=np.float32)
    Wv = np.asarray(Wv, dtype=np.float32)
    Wo = np.asarray(Wo, dtype=np.float32)
    active, mtiles = _classify_mask(mask)
    mt = (np.concatenate(mtiles, axis=1).astype(bf16)
          if mtiles else None)
    hsT = np.ascontiguousarray(hs.T)
    cosT, sinT = _rope_tables()
    maps = []
    for c in range(N_CORES):
        m = {
            "hsT": hsT,
            "wq": np.ascontiguousarray(
                Wq[:, c * G * DH:(c + 1) * G * DH] * np.float32(MULT)),
            "wk": np.ascontiguousarray(Wk[:, c * DH:(c + 1) * DH]),
            "wv": np.ascontiguousarray(Wv[:, c * DH:(c + 1) * DH]),
            "wo": np.ascontiguousarray(
                Wo[c * G * DH:(c + 1) * G * DH, :]).astype(bf16),
            "cosT": cosT,
            "sinT": sinT,
        }
        if mt is not None:
            m["maskm"] = np.ascontiguousarray(mt)
        maps.append(m)
    return active, mt, maps


def kernel(hidden_states, mask, Wq, Wk, Wv, Wo):
    from concourse.bass_utils import run_bass_kernel_spmd

    active, mt, maps = _in_maps(hidden_states, mask, Wq, Wk, Wv, Wo)
    key = active
    if key not in _CACHE:
        _CACHE[key] = _build(active, 0 if mt is None else mt.shape[1])
    nc = _CACHE[key]

    res = run_bass_kernel_spmd(nc, maps, list(range(N_CORES)))
    acc = np.zeros((S, D), dtype=np.float64)
    for c in range(N_CORES):
        acc += res.results[c]["out"]
    return acc.astype(np.float32).reshape(1, S, D)
